# revision 2
# baseline (speedup 1.0000x reference)
"""Trainium2 Bass kernel for nn_BilateralFilter (exact Gaussian bilateral filter).

Math (per reference):
  feats f_i in R^6 (scaled spatial zyx + scaled rgb), N = 12*24*24 = 6912
  sq[i,j] = |f_i - f_j|^2 ;  K = exp(-0.5*sq)
  out[c,j] = (sum_i q[c,i] K[i,j]) / (sum_i K[i,j] + eps)

Device strategy (8 cores, row-sharded over the N x N kernel):
  Each core owns J = N/8 = 864 output columns j.  A single PE matmul with an
  8-dim augmented feature contraction produces arg = f_i.f_j - 0.5|f_i|^2
  - 0.5|f_j|^2 = -0.5*sq directly in PSUM (tiles of 128 i x 864 j), ScalarE
  exponentiates PSUM->SBUF, and a second PE matmul contracts K against
  [q0, q1, ones] accumulating (3, 864) = (filtered0, filtered1, norm) in PSUM.
  The N x N kernel matrix never touches HBM.

Host does only O(N) layout/prep: building the augmented feature matrices and
the final (2 x N) normalization divide, matching the reference eps semantics.
"""

import os
import numpy as np

try:
    import concourse.bass as bass
except ImportError:  # fresh grading dir: repo not on sys.path
    import sys

    sys.path.insert(0, "/opt/trn_rl_repo")
    import concourse.bass as bass

import concourse.mybir as mybir
import concourse.tile as tile
from concourse import bacc
from concourse.bass_utils import run_bass_kernel_spmd

SIGMA_ALPHA = (5.0, 5.0, 5.0)
SIGMA_BETA = 0.3
EPS = float(np.finfo("float").eps)

D, H, W = 12, 24, 24
N = D * H * W  # 6912
M_CORES = 8
J = N // M_CORES  # 864 output columns per core
NT = N // 128  # 54 i-tiles
F = 8  # augmented feature dim
J_CHUNKS = [(0, 512), (512, 864)]  # matmul free-dim chunks, PSUM-bank aligned

# fp32 matmul is exact but 4 cycles/row on PE; float32r streams at 1 cycle/row.
# Set BILATERAL_MM_DTYPE=float32 to fall back to the exact path.
_MM_DTYPE_NAME = os.environ.get("BILATERAL_MM_DTYPE", "float32")

_BUILD_CACHE: dict[str, object] = {}


def _build_nc(mm_dtype_name: str):
    mm_dt = getattr(mybir.dt, mm_dtype_name)
    nc = bacc.Bacc(None, target_bir_lowering=False)

    a_dram = nc.dram_tensor("a_all", [F, N], mybir.dt.float32, kind="ExternalInput")
    b_dram = nc.dram_tensor("b_slab", [F, J], mybir.dt.float32, kind="ExternalInput")
    qa_dram = nc.dram_tensor("qa", [N, 3], mybir.dt.float32, kind="ExternalInput")
    out_dram = nc.dram_tensor("acc_out", [3, J], mybir.dt.float32, kind="ExternalOutput")

    with tile.TileContext(nc) as tc:
        with (
            tc.tile_pool(name="const", bufs=1) as const_pool,
            tc.tile_pool(name="kpool", bufs=4) as kpool,
            tc.tile_pool(name="gpsum", bufs=2, space="PSUM") as gpool,
            tc.tile_pool(name="apsum", bufs=1, space="PSUM") as apool,
            tc.tile_pool(name="opool", bufs=1) as opool,
        ):
            A = const_pool.tile([F, N], mybir.dt.float32)
            B = const_pool.tile([F, J], mybir.dt.float32)
            QA = const_pool.tile([128, NT * 3], mybir.dt.float32)
            nc.sync.dma_start(A[:], a_dram[:])
            nc.sync.dma_start(B[:], b_dram[:])
            nc.sync.dma_start(
                QA[:].rearrange("p (t c) -> p t c", c=3),
                qa_dram[:].rearrange("(t p) c -> p t c", p=128),
            )

            acc = apool.tile([3, J], mybir.dt.float32)
            for t in range(NT):
                g = gpool.tile([128, J], mybir.dt.float32)
                for j0, j1 in J_CHUNKS:
                    nc.tensor.matmul(
                        g[:, j0:j1],
                        A[:, t * 128 : (t + 1) * 128].bitcast(mm_dt),
                        B[:, j0:j1].bitcast(mm_dt),
                        start=True,
                        stop=True,
                    )
                k = kpool.tile([128, J], mybir.dt.float32)
                nc.scalar.activation(k[:], g[:], mybir.ActivationFunctionType.Exp)
                for j0, j1 in J_CHUNKS:
                    nc.tensor.matmul(
                        acc[:, j0:j1],
                        QA[:, t * 3 : t * 3 + 3].bitcast(mm_dt),
                        k[:, j0:j1].bitcast(mm_dt),
                        start=(t == 0),
                        stop=(t == NT - 1),
                    )

            out_sb = opool.tile([3, J], mybir.dt.float32)
            nc.vector.tensor_copy(out_sb[:], acc[:])
            nc.sync.dma_start(out_dram[:], out_sb[:])

    nc.compile()
    return nc


def _get_nc(mm_dtype_name: str):
    nc = _BUILD_CACHE.get(mm_dtype_name)
    if nc is None:
        nc = _build_nc(mm_dtype_name)
        _BUILD_CACHE[mm_dtype_name] = nc
    return nc


def _host_prep(q_in, image, v_alpha, v_beta):
    """Augmented feature matrices (fp32, O(N) work only)."""
    q_in = np.asarray(q_in, dtype=np.float32)
    image = np.asarray(image, dtype=np.float32)
    v_alpha = np.asarray(v_alpha, dtype=np.float32)
    v_beta = np.asarray(v_beta, dtype=np.float32)

    z = np.arange(D, dtype=np.float32)[:, None, None]
    y = np.arange(H, dtype=np.float32)[None, :, None]
    x = np.arange(W, dtype=np.float32)[None, None, :]
    shp = (D, H, W)
    zz = np.broadcast_to(v_alpha[0] * z / np.float32(SIGMA_ALPHA[0]), shp)
    xx = np.broadcast_to(v_alpha[1] * x / np.float32(SIGMA_ALPHA[1]), shp)
    yy = np.broadcast_to(v_alpha[2] * y / np.float32(SIGMA_ALPHA[2]), shp)
    xyz = np.stack([zz, yy, xx], axis=3)
    rgb = v_beta * np.transpose(image, (1, 2, 3, 0)) / np.float32(SIGMA_BETA)
    feats = np.concatenate([xyz, rgb], axis=3).reshape(-1, 6).astype(np.float32)

    # Center each feature dim: |f_i - f_j| is translation invariant, smaller
    # magnitudes mean less cancellation in the PE accumulation.
    feats = feats - (feats.min(axis=0) + feats.max(axis=0)) * np.float32(0.5)

    s = np.einsum("nf,nf->n", feats, feats).astype(np.float32)

    a_all = np.empty((F, N), dtype=np.float32)
    a_all[0:6] = feats.T
    a_all[6] = -0.5 * s
    a_all[7] = 1.0

    b_full = np.empty((F, N), dtype=np.float32)
    b_full[0:6] = feats.T
    b_full[6] = 1.0
    b_full[7] = -0.5 * s

    qa = np.empty((N, 3), dtype=np.float32)
    qa[:, 0] = q_in[0].reshape(-1)
    qa[:, 1] = q_in[1].reshape(-1)
    qa[:, 2] = 1.0
    return a_all, b_full, qa


def kernel(q_in, image, v_alpha, v_beta):
    a_all, b_full, qa = _host_prep(q_in, image, v_alpha, v_beta)

    nc = _get_nc(_MM_DTYPE_NAME)
    in_maps = [
        {
            "a_all": a_all,
            "b_slab": np.ascontiguousarray(b_full[:, c * J : (c + 1) * J]),
            "qa": qa,
        }
        for c in range(M_CORES)
    ]
    res = run_bass_kernel_spmd(nc, in_maps, core_ids=list(range(M_CORES)))

    acc = np.concatenate([res.results[c]["acc_out"] for c in range(M_CORES)], axis=1)
    filtered = acc[0:2]
    norm = acc[2]
    out = filtered / (norm[None, :] + EPS)
    return out.reshape(2, D, H, W).astype(np.float32)
